# revision 26
# baseline (speedup 1.0000x reference)
"""Trainium2 Bass kernel: Conv2d(1->64, k=7, valid) on data [32,1,224,224] f32.

Data-parallel over batch (4 images per core on 8 cores).  Per core the conv
is an im2col matmul structured around the PE's 1-column/cycle stream rate:

1. Paired output rows: each matmul column computes 2 output rows x 64
   channels (M=128, K=56 taps = 8 ky' x 7 kx).  Column (pair p, x) needs
   taps in[r0+2p+ky', x+kx].  Streamed columns per core = out pixels / 2.
2. Two tiles share the 128 partitions (tap blocks at 0:56 and 64:120,
   tile_position (0,0)/(64,0)) so LDWEIGHTS double-buffers across tiles.
3. im2col is ONE SWDGE DMA per (tile, row-parity): source is 4 slab
   copies on consecutive partitions (ky2 = partition step), each 15
   stride-2 rows of the image, read with overlapping AP
   [[p_stride,4],[1,7],[1,3144]] -> 28 dst partitions, contiguous 6.3KB
   runs.  Tap index = par*28 + ky2*7 + kx; ky' = 2*ky2 + par.

PSUM: [128,1024] 2-bank tiles for chunk pairs + [128,512] for the odd 7th
chunk.  DVE/ACT copies compact 224-wide pairs to 218 valid cols in ob, so
the fp16 stores are fully contiguous (one DMA per tile, 6.1KB runs, sync
HWDGE).  Output DRAM layout [IPC, 2(parity), 64, 109, 218]; the host
re-interleaves even/odd rows and casts to fp32.
"""

import numpy as np

B = 32            # full batch
OC = 64           # out channels
KS = 7            # kernel size
H = 224           # input H=W
OH = 218          # valid output rows/cols
NCORES = 8
IPC = B // NCORES  # images per core

BLK = 28          # output rows per tile
NBLK = 8          # tiles per image
NTILES = IPC * NBLK       # 32 tiles per core
SLAB_ROWS = 15            # rows per (parity, ky2) slab copy
SLAB = SLAB_ROWS * H      # 3360
SLABP = SLAB + 8          # padded slab pitch
SLOTW = 2 * SLABP         # two slab slots per partition
NCOL = 3144               # im2col cols per tile (14*224 + 8 pad)
CW = 448                  # matmul chunk width (2 pairs x 224)
NPAIRS = 14               # row pairs per tile
OBW = NPAIRS * OH         # 3052 compact ob cols
NPIMG = 109               # row pairs per image (218/2)

_CACHE = {}


def _slab_pos(t, par):
    # slab copies for (tile, parity, ky2) sit at partition stride 4 = one
    # SBUF cluster apart (port = ((p>>2)&7)<<1 | p>>6), so each im2col DMA
    # reads via 4 distinct AXI ports and a group's 4 DMAs cover all 16.
    k = 2 * t + par
    b0, b1 = k & 1, (k >> 1) & 1
    j, b4, slot = (k >> 2) & 3, (k >> 4) & 1, k >> 5
    return 16 * b1 + j + 64 * b0 + 32 * b4, slot


def _build():
    import concourse.bass as bass
    import concourse.mybir as mybir
    import concourse.tile as tile
    from concourse import bacc

    nc = bacc.Bacc("TRN2", target_bir_lowering=False, debug=False)

    xb = nc.dram_tensor("xb", [128, SLOTW], mybir.dt.float16,
                        kind="ExternalInput")
    wa = nc.dram_tensor("wa", [128, 128], mybir.dt.float16,
                        kind="ExternalInput")
    out = nc.dram_tensor("out", [IPC, 2, OC, NPIMG, OH], mybir.dt.float16,
                         kind="ExternalOutput")

    with tile.TileContext(nc) as tc:
        with (
            tc.tile_pool(name="src", bufs=1) as src_pool,
            tc.tile_pool(name="wp", bufs=1) as w_pool,
            tc.tile_pool(name="i2c", bufs=8) as i2c_pool,
            tc.tile_pool(name="ob", bufs=6) as ob_pool,
            tc.tile_pool(name="ps2", bufs=4, space="PSUM") as ps2_pool,
        ):
            oap = out.ap()
            srct = src_pool.tile([128, SLOTW], mybir.dt.float16)
            wta = w_pool.tile([128, 128], mybir.dt.float16)

            p_stride = srct.ap[0][0]  # partition pitch in elements

            nc.sync.dma_start(out=wta[:, :], in_=wa[:, :])
            # load slab bands in group-usage order: groups 0-3/8-11 read
            # partitions [0:32]+[64:96] (b4=0), groups 4-7/12-15 the +32
            # bands, so early groups unblock at ~half the load.
            for lo in (0, 64, 32, 96):
                nc.sync.dma_start(out=srct[lo:lo + 32, :],
                                  in_=xb[lo:lo + 32, :])

            # software-pipelined emission: issue im2col DMAs PREFETCH
            # groups ahead so POOL's in-order stream never stalls emission.
            PREFETCH = 7
            NGRP = NTILES // 2
            i2c_tiles = {}

            def issue_i2c(g):
                i2c = i2c_pool.tile([128, NCOL], mybir.dt.float16,
                                    tag="i2c", name=f"i2c{g}")
                for u in range(2):
                    t = 2 * g + u
                    for par in range(2):
                        p0, slot = _slab_pos(t, par)
                        src = bass.AP(
                            tensor=srct.tensor,
                            offset=srct.offset + p0 * p_stride
                            + slot * SLABP,
                            ap=[[4 * p_stride, 4], [1, 7], [1, NCOL]],
                        )
                        b0 = 64 * u + 28 * par
                        nc.gpsimd.dma_start(
                            out=i2c[b0:b0 + 28, :], in_=src,
                            single_packet=True)
                i2c_tiles[g] = i2c

            for g in range(min(PREFETCH, NGRP)):
                issue_i2c(g)

            ncopy = 0
            for g in range(NGRP):
                if g + PREFETCH < NGRP:
                    issue_i2c(g + PREFETCH)
                i2c = i2c_tiles.pop(g)

                tiles = []
                for u in range(2):
                    t = 2 * g + u
                    img, blk = divmod(t, NBLK)
                    tiles.append({
                        "img": img, "blk": blk, "u": u,
                        "full": blk < NBLK - 1,
                        "ob": ob_pool.tile([128, OBW + 4], mybir.dt.float16,
                                           tag="ob", name=f"ob{t}"),
                        "o0": 0,
                        "rhs": i2c[64 * u:64 * u + 56, :],
                        "lh": wta[64 * u:64 * u + 56, :],
                        "tp": (64 * u, 0),
                    })

                # chunk-major emission, rotating the two tile positions so
                # LDWEIGHTS for one overlaps the other's streaming matmul.
                for pc in range(4):
                    pss = {}
                    for ti in tiles:
                        if pc == 3 and not ti["full"]:
                            continue  # pairs 12,13 are garbage for blk 7
                        pss[ti["u"]] = ps2_pool.tile(
                            [128, 1024], mybir.dt.float32, tag="ps2",
                            name=f"ps{g}_{pc}_{ti['u']}")
                    nch = 2 if pc < 3 else 1
                    for ti in tiles:
                        if ti["u"] not in pss:
                            continue
                        for h in range(nch):
                            c0 = (2 * pc + h) * CW
                            nc.tensor.matmul(
                                pss[ti["u"]][:, 512 * h:512 * h + CW],
                                ti["lh"], ti["rhs"][:, c0:c0 + CW],
                                start=True, stop=True,
                                tile_position=ti["tp"])
                    for ti in tiles:
                        if ti["u"] not in pss:
                            continue
                        ps = pss[ti["u"]]
                        ob = ti["ob"]
                        o0 = ti["o0"]
                        # compact copies: drop the 6 garbage cols per 224
                        if pc < 3:
                            if pc == 2 and not ti["full"]:
                                # pairs 8,9 full; pair 10 valid; 11 garbage
                                csrc = bass.AP(
                                    tensor=ps.tensor, offset=ps.offset,
                                    ap=[list(ps.ap[0]), [224, 2], [1, OH]])
                                nc.vector.tensor_copy(
                                    ob[:, o0 + 8 * OH:o0 + 10 * OH], csrc)
                                csrc = bass.AP(
                                    tensor=ps.tensor,
                                    offset=ps.offset + 512,
                                    ap=[list(ps.ap[0]), [1, OH]])
                                nc.scalar.copy(
                                    ob[:, o0 + 10 * OH:o0 + 11 * OH], csrc)
                                continue
                            csrc = bass.AP(
                                tensor=ps.tensor, offset=ps.offset,
                                ap=[list(ps.ap[0]), [512, 2], [224, 2],
                                    [1, OH]])
                            cdst = ob[:, o0 + 4 * pc * OH:
                                      o0 + (4 * pc + 4) * OH]
                        else:
                            csrc = bass.AP(
                                tensor=ps.tensor, offset=ps.offset,
                                ap=[list(ps.ap[0]), [224, 2], [1, OH]])
                            cdst = ob[:, o0 + 12 * OH:o0 + 14 * OH]
                        if ncopy % 2 == 0:
                            nc.vector.tensor_copy(cdst, csrc)
                        else:
                            nc.scalar.copy(cdst, csrc)
                        ncopy += 1

                # fp16 stores: one fully contiguous DMA per tile (sync HWDGE)
                for ti in tiles:
                    npair = NPAIRS if ti["full"] else NPIMG - NPAIRS * ti["blk"]
                    dst = bass.AP(
                        tensor=oap.tensor,
                        offset=oap.offset
                        + (ti["img"] * 2 * OC * NPIMG
                           + NPAIRS * ti["blk"]) * OH,
                        ap=[[NPIMG * OH, 128], [1, npair * OH]],
                    )
                    nc.sync.dma_start(out=dst, in_=ti["ob"][:, :npair * OH])

    nc.compile()
    return nc


def _prep_inputs(data, weight):
    d16 = np.asarray(data).reshape(B, H, H).astype(np.float16)
    dpad = np.zeros((B, 236, H), dtype=np.float16)
    dpad[:, :H, :] = d16

    w = np.asarray(weight).reshape(OC, KS, KS).astype(np.float32)
    wa56 = np.zeros((56, 128), dtype=np.float32)
    for par in range(2):
        for ky2 in range(4):
            kyp = 2 * ky2 + par
            for kx in range(KS):
                idx = par * 28 + ky2 * KS + kx
                if kyp <= 6:
                    wa56[idx, :OC] = w[:, kyp, kx]
                if 1 <= kyp <= 7:
                    wa56[idx, OC:] = w[:, kyp - 1, kx]
    wa = np.zeros((128, 128), dtype=np.float16)
    wa[0:56] = wa56
    wa[64:120] = wa56

    in_maps = []
    for core in range(NCORES):
        xb = np.zeros((128, SLOTW), dtype=np.float16)
        for t in range(NTILES):
            img, blk = divmod(t, NBLK)
            gimg = core * IPC + img
            r0 = BLK * blk
            for par in range(2):
                p0, slot = _slab_pos(t, par)
                for ky2 in range(4):
                    rs = r0 + par + 2 * ky2
                    xb[p0 + 4 * ky2,
                       slot * SLABP: slot * SLABP + SLAB] = \
                        dpad[gimg, rs: rs + 2 * SLAB_ROWS: 2, :].ravel()
        in_maps.append({"xb": xb, "wa": wa})
    return in_maps


def kernel(data, weight):
    from concourse.bass_utils import run_bass_kernel_spmd

    if "nc" not in _CACHE:
        _CACHE["nc"] = _build()
    nc = _CACHE["nc"]

    in_maps = _prep_inputs(data, weight)
    res = run_bass_kernel_spmd(nc, in_maps, core_ids=list(range(NCORES)))
    outs = [r["out"] for r in res.results]  # [IPC, 2, 64, 109, 218] each
    full = np.concatenate(outs, axis=0)     # [32, 2, 64, 109, 218]
    final = np.empty((B, OC, OH, OH), dtype=np.float32)
    final[:, :, 0::2, :] = full[:, 0]
    final[:, :, 1::2, :] = full[:, 1]
    return final


# revision 27
# speedup vs baseline: 1.0180x; 1.0180x over previous
"""Trainium2 Bass kernel: Conv2d(1->64, k=7, valid) on data [32,1,224,224] f32.

Data-parallel over batch (4 images per core on 8 cores).  Per core the conv
is an im2col matmul structured around the PE's 1-column/cycle stream rate:

1. Paired output rows: each matmul column computes 2 output rows x 64
   channels (M=128, K=56 taps = 8 ky' x 7 kx).  Column (pair p, x) needs
   taps in[r0+2p+ky', x+kx].  Streamed columns per core = out pixels / 2.
2. Two tiles share the 128 partitions (tap blocks at 0:56 and 64:120,
   tile_position (0,0)/(64,0)) so LDWEIGHTS double-buffers across tiles.
3. im2col is ONE SWDGE DMA per (tile, row-parity): source is 4 slab
   copies on consecutive partitions (ky2 = partition step), each 15
   stride-2 rows of the image, read with overlapping AP
   [[p_stride,4],[1,7],[1,3144]] -> 28 dst partitions, contiguous 6.3KB
   runs.  Tap index = par*28 + ky2*7 + kx; ky' = 2*ky2 + par.

PSUM: [128,1024] 2-bank tiles for chunk pairs + [128,512] for the odd 7th
chunk.  DVE/ACT copies compact 224-wide pairs to 218 valid cols in ob, so
the fp16 stores are fully contiguous (one DMA per tile, 6.1KB runs, sync
HWDGE).  Output DRAM layout [IPC, 2(parity), 64, 109, 218]; the host
re-interleaves even/odd rows and casts to fp32.
"""

import numpy as np

B = 32            # full batch
OC = 64           # out channels
KS = 7            # kernel size
H = 224           # input H=W
OH = 218          # valid output rows/cols
NCORES = 8
IPC = B // NCORES  # images per core

BLK = 28          # output rows per tile
NBLK = 8          # tiles per image
NTILES = IPC * NBLK       # 32 tiles per core
SLAB_ROWS = 15            # rows per (parity, ky2) slab copy
SLAB = SLAB_ROWS * H      # 3360
SLABP = SLAB + 8          # padded slab pitch
SLOTW = 2 * SLABP         # two slab slots per partition
NCOL = 3144               # im2col cols per tile (14*224 + 8 pad)
CW = 448                  # matmul chunk width (2 pairs x 224)
NPAIRS = 14               # row pairs per tile
OBW = NPAIRS * OH         # 3052 compact ob cols
NPIMG = 109               # row pairs per image (218/2)

_CACHE = {}


def _slab_pos(t, par):
    # slab copies for (tile, parity, ky2) sit at partition stride 4 = one
    # SBUF cluster apart (port = ((p>>2)&7)<<1 | p>>6), so each im2col DMA
    # reads via 4 distinct AXI ports and a group's 4 DMAs cover all 16.
    k = 2 * t + par
    b0, b1 = k & 1, (k >> 1) & 1
    j, b4, slot = (k >> 2) & 3, (k >> 4) & 1, k >> 5
    return 16 * b1 + j + 64 * b0 + 32 * b4, slot


def _build():
    import concourse.bass as bass
    import concourse.mybir as mybir
    import concourse.tile as tile
    from concourse import bacc

    nc = bacc.Bacc("TRN2", target_bir_lowering=False, debug=False)

    xb = nc.dram_tensor("xb", [128, SLOTW], mybir.dt.float16,
                        kind="ExternalInput")
    wa = nc.dram_tensor("wa", [128, 128], mybir.dt.float16,
                        kind="ExternalInput")
    out = nc.dram_tensor("out", [IPC, 2, OC, NPIMG, OH], mybir.dt.float16,
                         kind="ExternalOutput")

    with tile.TileContext(nc) as tc:
        with (
            tc.tile_pool(name="src", bufs=1) as src_pool,
            tc.tile_pool(name="wp", bufs=1) as w_pool,
            tc.tile_pool(name="i2c", bufs=6) as i2c_pool,
            tc.tile_pool(name="ob", bufs=6) as ob_pool,
            tc.tile_pool(name="ps2", bufs=4, space="PSUM") as ps2_pool,
        ):
            oap = out.ap()
            srct = src_pool.tile([128, SLOTW], mybir.dt.float16)
            wta = w_pool.tile([128, 128], mybir.dt.float16)

            p_stride = srct.ap[0][0]  # partition pitch in elements

            nc.sync.dma_start(out=wta[:, :], in_=wa[:, :])
            nc.sync.dma_start(out=srct[:, :], in_=xb[:, :])

            # software-pipelined emission: issue im2col DMAs PREFETCH
            # groups ahead so POOL's in-order stream never stalls emission.
            PREFETCH = 5
            NGRP = NTILES // 2
            i2c_tiles = {}

            def issue_i2c(g):
                i2c = i2c_pool.tile([128, NCOL], mybir.dt.float16,
                                    tag="i2c", name=f"i2c{g}")
                for u in range(2):
                    t = 2 * g + u
                    for par in range(2):
                        p0, slot = _slab_pos(t, par)
                        src = bass.AP(
                            tensor=srct.tensor,
                            offset=srct.offset + p0 * p_stride
                            + slot * SLABP,
                            ap=[[4 * p_stride, 4], [1, 7], [1, NCOL]],
                        )
                        b0 = 64 * u + 28 * par
                        nc.gpsimd.dma_start(
                            out=i2c[b0:b0 + 28, :], in_=src,
                            single_packet=True)
                i2c_tiles[g] = i2c

            for g in range(min(PREFETCH, NGRP)):
                issue_i2c(g)

            ncopy = 0
            for g in range(NGRP):
                if g + PREFETCH < NGRP:
                    issue_i2c(g + PREFETCH)
                i2c = i2c_tiles.pop(g)

                tiles = []
                for u in range(2):
                    t = 2 * g + u
                    img, blk = divmod(t, NBLK)
                    tiles.append({
                        "img": img, "blk": blk, "u": u,
                        "full": blk < NBLK - 1,
                        "ob": ob_pool.tile([128, OBW + 4], mybir.dt.float16,
                                           tag="ob", name=f"ob{t}"),
                        "o0": 0,
                        "rhs": i2c[64 * u:64 * u + 56, :],
                        "lh": wta[64 * u:64 * u + 56, :],
                        "tp": (64 * u, 0),
                    })

                # chunk-major emission, rotating the two tile positions so
                # LDWEIGHTS for one overlaps the other's streaming matmul.
                for pc in range(4):
                    pss = {}
                    for ti in tiles:
                        if pc == 3 and not ti["full"]:
                            continue  # pairs 12,13 are garbage for blk 7
                        pss[ti["u"]] = ps2_pool.tile(
                            [128, 1024], mybir.dt.float32, tag="ps2",
                            name=f"ps{g}_{pc}_{ti['u']}")
                    nch = 2 if pc < 3 else 1
                    for ti in tiles:
                        if ti["u"] not in pss:
                            continue
                        for h in range(nch):
                            c0 = (2 * pc + h) * CW
                            nc.tensor.matmul(
                                pss[ti["u"]][:, 512 * h:512 * h + CW],
                                ti["lh"], ti["rhs"][:, c0:c0 + CW],
                                start=True, stop=True,
                                tile_position=ti["tp"])
                    for ti in tiles:
                        if ti["u"] not in pss:
                            continue
                        ps = pss[ti["u"]]
                        ob = ti["ob"]
                        o0 = ti["o0"]
                        # compact copies: drop the 6 garbage cols per 224
                        if pc < 3:
                            if pc == 2 and not ti["full"]:
                                # pairs 8,9 full; pair 10 valid; 11 garbage
                                csrc = bass.AP(
                                    tensor=ps.tensor, offset=ps.offset,
                                    ap=[list(ps.ap[0]), [224, 2], [1, OH]])
                                nc.vector.tensor_copy(
                                    ob[:, o0 + 8 * OH:o0 + 10 * OH], csrc)
                                csrc = bass.AP(
                                    tensor=ps.tensor,
                                    offset=ps.offset + 512,
                                    ap=[list(ps.ap[0]), [1, OH]])
                                nc.scalar.copy(
                                    ob[:, o0 + 10 * OH:o0 + 11 * OH], csrc)
                                continue
                            csrc = bass.AP(
                                tensor=ps.tensor, offset=ps.offset,
                                ap=[list(ps.ap[0]), [512, 2], [224, 2],
                                    [1, OH]])
                            cdst = ob[:, o0 + 4 * pc * OH:
                                      o0 + (4 * pc + 4) * OH]
                        else:
                            csrc = bass.AP(
                                tensor=ps.tensor, offset=ps.offset,
                                ap=[list(ps.ap[0]), [224, 2], [1, OH]])
                            cdst = ob[:, o0 + 12 * OH:o0 + 14 * OH]
                        if ncopy % 2 == 0:
                            nc.vector.tensor_copy(cdst, csrc)
                        else:
                            nc.scalar.copy(cdst, csrc)
                        ncopy += 1

                # fp16 stores: one fully contiguous DMA per tile (sync HWDGE)
                for ti in tiles:
                    npair = NPAIRS if ti["full"] else NPIMG - NPAIRS * ti["blk"]
                    dst = bass.AP(
                        tensor=oap.tensor,
                        offset=oap.offset
                        + (ti["img"] * 2 * OC * NPIMG
                           + NPAIRS * ti["blk"]) * OH,
                        ap=[[NPIMG * OH, 128], [1, npair * OH]],
                    )
                    eng = nc.sync if ti["u"] == 0 else nc.scalar
                    eng.dma_start(out=dst, in_=ti["ob"][:, :npair * OH])

    nc.compile()
    return nc


def _prep_inputs(data, weight):
    d16 = np.asarray(data).reshape(B, H, H).astype(np.float16)
    dpad = np.zeros((B, 236, H), dtype=np.float16)
    dpad[:, :H, :] = d16

    w = np.asarray(weight).reshape(OC, KS, KS).astype(np.float32)
    wa56 = np.zeros((56, 128), dtype=np.float32)
    for par in range(2):
        for ky2 in range(4):
            kyp = 2 * ky2 + par
            for kx in range(KS):
                idx = par * 28 + ky2 * KS + kx
                if kyp <= 6:
                    wa56[idx, :OC] = w[:, kyp, kx]
                if 1 <= kyp <= 7:
                    wa56[idx, OC:] = w[:, kyp - 1, kx]
    wa = np.zeros((128, 128), dtype=np.float16)
    wa[0:56] = wa56
    wa[64:120] = wa56

    in_maps = []
    for core in range(NCORES):
        xb = np.zeros((128, SLOTW), dtype=np.float16)
        for t in range(NTILES):
            img, blk = divmod(t, NBLK)
            gimg = core * IPC + img
            r0 = BLK * blk
            for par in range(2):
                p0, slot = _slab_pos(t, par)
                for ky2 in range(4):
                    rs = r0 + par + 2 * ky2
                    xb[p0 + 4 * ky2,
                       slot * SLABP: slot * SLABP + SLAB] = \
                        dpad[gimg, rs: rs + 2 * SLAB_ROWS: 2, :].ravel()
        in_maps.append({"xb": xb, "wa": wa})
    return in_maps


def kernel(data, weight):
    from concourse.bass_utils import run_bass_kernel_spmd

    if "nc" not in _CACHE:
        _CACHE["nc"] = _build()
    nc = _CACHE["nc"]

    in_maps = _prep_inputs(data, weight)
    res = run_bass_kernel_spmd(nc, in_maps, core_ids=list(range(NCORES)))
    outs = [r["out"] for r in res.results]  # [IPC, 2, 64, 109, 218] each
    full = np.concatenate(outs, axis=0)     # [32, 2, 64, 109, 218]
    final = np.empty((B, OC, OH, OH), dtype=np.float32)
    final[:, :, 0::2, :] = full[:, 0]
    final[:, :, 1::2, :] = full[:, 1]
    return final
